# revision 33
# baseline (speedup 1.0000x reference)
"""MoE (top-2 of 8 experts, SwiGLU) Trainium2 kernel — fused bf16 single pass.

Strategy (expert-parallel over 8 NeuronCores):
  * Host: router GEMM + top-2 + sigmoid gates in numpy (selection verified to
    match the jax fp32 reference on these inputs), then gather each expert's
    tokens into a transposed, capacity-padded bf16 buffer xT_e [H, C]. One
    expert per core, capacity C = max_e count_e.
  * Device (SPMD, per core): all three weight matrices live in SBUF as bf16
    (12 MB total), x and the gates are SBUF-resident too.  Tokens are
    processed in segments of <=512 (the PSUM free-dim limit); for each
    segment the SwiGLU intermediate h = silu(x@Wg) * (x@Wu) is produced
    i-block by i-block into SBUF (bf16) and immediately consumed by the
    down-projection y = (h@Wd) * gate — h never leaves the chip.
    The down-projection runs hb-outer so only 2 PSUM banks are needed and
    eviction is progressive (short kernel tail).
  * bf16 matmuls stream at the same 1 cycle/row as fp32r but halve SBUF and
    HBM traffic and enable fast weight loads; accumulation is fp32 in PSUM.
    The measured output error vs the fp64 reference is ~3e-3.
  * A burst of warm-up matmuls on the first x tile runs while weights are
    still streaming in, so the PE's HAM clock gate reaches full rate before
    the real matmul stream starts.
  * Host: out[idx_e] += yT_e[:, :n_e].T  (indices within one expert are
    unique, so fancy-index += is safe).
"""

import os
import numpy as np
import ml_dtypes

T, H, I, E, TOPK = 8192, 1024, 2048, 8, 2
NCORES = 8
PB = 128
KB = H // PB   # 8 contraction blocks over H
IB = I // PB   # 16 blocks over I
HB = H // PB   # 8 output blocks over H

# Wg/Wu i-block chunks (need-ordered streaming, one DMA per chunk).
# Host re-lays Wg/Wu as WgR[r, (ib*KB + k)*PB + c] = Wg[k*PB + r, ib*PB + c]
# so that any i-block range for ALL k-blocks is one contiguous DMA.
WCH = [(0, 1), (1, 2), (2, 3), (3, 4), (4, 8), (8, 12), (12, 16)]
DCH = [(0, 8), (8, 16)]   # Wd i-block chunks (WdR layout, see below)

_compiled = {}
last_results = None  # BassKernelResults of the most recent run (for test harness)


def _tsegs(C):
    """Split C into segments of width 256..512, with the first 512 split in
    two so the matmul stream can start on fewer landed bytes."""
    widths = [256, 256]
    rem = C - 512
    while rem >= 768:
        widths.append(512)
        rem -= 512
    if rem == 0:
        pass
    elif rem <= 512:
        widths.append(rem)
    else:
        widths.append(rem - 256)
        widths.append(256)
    segs = []
    t0 = 0
    for tb in widths:
        segs.append((t0, tb))
        t0 += tb
    return segs


def _build(C):
    import concourse.bacc as bacc
    import concourse.mybir as mybir
    import concourse.tile as tile
    from contextlib import ExitStack

    fp32 = mybir.dt.float32
    bf16 = mybir.dt.bfloat16
    AF = mybir.ActivationFunctionType

    segs = _tsegs(C)
    XSW = 512   # columns covered by the early xs0 tiles (first two segments)

    nc = bacc.Bacc("TRN2", target_bir_lowering=False, debug=False,
                   num_devices=NCORES)
    xT = nc.dram_tensor("xT", [H, C], bf16, kind="ExternalInput").ap()
    gm = nc.dram_tensor("gm", [PB, C], fp32, kind="ExternalInput").ap()
    Wg = nc.dram_tensor("Wg", [PB, IB * KB * PB], bf16,
                        kind="ExternalInput").ap()
    Wu = nc.dram_tensor("Wu", [PB, IB * KB * PB], bf16,
                        kind="ExternalInput").ap()
    Wd = nc.dram_tensor("Wd", [PB, IB * H], bf16, kind="ExternalInput").ap()
    yT = nc.dram_tensor("yT", [H, C], fp32, kind="ExternalOutput").ap()

    with tile.TileContext(nc) as tc, ExitStack() as st:
        wp = st.enter_context(tc.tile_pool(name="wp", bufs=1))
        hp = st.enter_context(tc.tile_pool(name="hp", bufs=2))
        ev1 = st.enter_context(tc.tile_pool(name="ev1", bufs=2))
        ev2 = st.enter_context(tc.tile_pool(name="ev2", bufs=3))
        ps1 = st.enter_context(tc.tile_pool(name="ps1", bufs=2, space="PSUM"))
        ps2 = st.enter_context(tc.tile_pool(name="ps2", bufs=4, space="PSUM"))

        # ---- load issue order.  The critical stream (x seg0, then Wg/Wu in
        # i-block need-order) is split between the sync and gpsimd queues;
        # everything needed later (gates, Wd, x remainder) goes on the scalar
        # queue, paced behind the per-i-block silu ops so it cannot steal
        # bandwidth from the critical window. ----
        # Warm-up matmuls on a memset scratch tile (no DMA dependency): the
        # PE is busy from ~7us — right after the framework preamble — so the
        # HAM clock gate reaches 8/8 before the real stream starts, and the
        # PE has work while the first weight chunks land.  They write
        # rotating ps2 slots, long retired before phase 2 reuses them.
        wscr = wp.tile([PB, 512], bf16, name="wscr")
        nc.gpsimd.memset(wscr[:], 0.0)
        for i in range(12):
            pwarm = ps2.tile([PB, 512], fp32, tag="py", name="py")
            nc.tensor.matmul(pwarm[:], wscr[:, 0:PB], wscr[:],
                             start=True, stop=True)

        # Segment-0 x: per-k tiles spread over all three queues so arrival
        # granularity is fine (chains pipeline with landings).
        xq = [nc.sync, nc.gpsimd, nc.scalar]
        xs0 = []
        for k in range(KB):
            t = wp.tile([PB, XSW], bf16, name=f"xs0_{k}")
            xq[k % 3].dma_start(out=t[:], in_=xT[k * PB:(k + 1) * PB, 0:XSW])
            xs0.append(t)

        # Wg/Wu in need-ordered i-block chunks, one DMA each, alternating
        # sync/gpsimd so both queues carry half of the critical stream.
        IBW = KB * PB   # column span of one i-block in the WgR/WuR layout
        wgt, wut = [], []
        for c, (a, b) in enumerate(WCH):
            qa, qb = (nc.sync, nc.gpsimd) if c % 2 == 0 \
                else (nc.gpsimd, nc.sync)
            t = wp.tile([PB, (b - a) * IBW], bf16, name=f"wg{c}")
            qa.dma_start(out=t[:], in_=Wg[:, a * IBW:b * IBW])
            wgt.append(t)
            t = wp.tile([PB, (b - a) * IBW], bf16, name=f"wu{c}")
            qb.dma_start(out=t[:], in_=Wu[:, a * IBW:b * IBW])
            wut.append(t)

        # Late loads (gates, Wd, x remainder): issued at the BACK of the sync
        # and gpsimd queues.  In-queue FIFO ordering paces their transfers
        # behind the critical Wg/Wu stream — the Tile scheduler would hoist
        # them if they sat dep-free on an otherwise-busy engine.
        gt = wp.tile([PB, C], fp32, name="gt")
        wdt = [wp.tile([PB, (b - a) * H], bf16, name=f"wd{c}")
               for c, (a, b) in enumerate(DCH)]
        xr = [wp.tile([PB, C - XSW], bf16, name=f"xr{k}") for k in range(KB)] \
            if C > XSW else []
        nc.gpsimd.dma_start(out=gt[:], in_=gm[:])
        for c, (a, b) in enumerate(DCH):
            q = nc.sync if c % 2 == 0 else nc.gpsimd
            q.dma_start(out=wdt[c][:], in_=Wd[:, a * H:b * H])
        for k in range(len(xr)):
            q = nc.sync if k % 2 == 0 else nc.gpsimd
            q.dma_start(out=xr[k][:], in_=xT[k * PB:(k + 1) * PB, XSW:C])

        def _chunk(ch, ib):
            for j, (a, b) in enumerate(ch):
                if ib < b:
                    return j, ib - a
            raise AssertionError

        def wg_sl(k, ib):
            j, off = _chunk(WCH, ib)
            return wgt[j][:, (off * KB + k) * PB:(off * KB + k) * PB + PB]

        def wu_sl(k, ib):
            j, off = _chunk(WCH, ib)
            return wut[j][:, (off * KB + k) * PB:(off * KB + k) * PB + PB]

        def wd_sl(ib, hb):
            j, off = _chunk(DCH, ib)
            return wdt[j][:, off * H + hb * PB:off * H + hb * PB + PB]

        def x_sl(k, t0, w):
            if t0 >= XSW:
                return xr[k][:, t0 - XSW:t0 - XSW + w]
            return xs0[k][:, t0:t0 + w]

        for si, (t0, w) in enumerate(segs):
            last_seg = si == len(segs) - 1
            hts = []
            for ib in range(IB):
                pg = ps1.tile([PB, w], fp32, tag="pg", name="pg")
                pu = ps1.tile([PB, w], fp32, tag="pu", name="pu")
                for k in range(KB):
                    nc.tensor.matmul(pg[:], wg_sl(k, ib), x_sl(k, t0, w),
                                     start=(k == 0), stop=(k == KB - 1))
                for k in range(KB):
                    nc.tensor.matmul(pu[:], wu_sl(k, ib), x_sl(k, t0, w),
                                     start=(k == 0), stop=(k == KB - 1))
                sg = ev1.tile([PB, w], fp32, tag="sg", name="sg")
                nc.scalar.activation(sg[:], pg[:], AF.Silu)
                hh = hp.tile([PB, w], bf16, tag=f"h{ib}", name=f"h{ib}")
                nc.vector.tensor_mul(hh[:], sg[:], pu[:])
                hts.append(hh)

            for hb in range(HB):
                py = ps2.tile([PB, w], fp32, tag="py", name="py")
                for ib in range(IB):
                    nc.tensor.matmul(py[:], wd_sl(ib, hb), hts[ib][:],
                                     start=(ib == 0), stop=(ib == IB - 1))
                yt = ev2.tile([PB, w], fp32, tag="yt", name="yt")
                nc.vector.tensor_mul(yt[:], py[:], gt[:, t0:t0 + w])
                eng = nc.sync if last_seg else nc.gpsimd
                eng.dma_start(out=yT[hb * PB:(hb + 1) * PB, t0:t0 + w],
                              in_=yt[:])
    nc.compile()
    return nc


def _route(x, Wr, br):
    """Replicate the reference's fp32 router bit-compatibly on host."""
    logits = x @ Wr + br                       # fp32 GEMM
    order = np.argsort(-logits, axis=1, kind="stable")  # ties -> lowest index
    topk_idx = order[:, :TOPK]
    topk_vals = np.take_along_axis(logits, topk_idx, axis=1)
    g = 1.0 / (1.0 + np.exp(-topk_vals.astype(np.float32)))
    g = g / (np.sum(g, axis=-1, keepdims=True) + 1e-10)
    return topk_idx, g.astype(np.float32)


def kernel(x, Wr, br, Wg, Wu, Wd):
    global last_results
    from concourse.bass_utils import run_bass_kernel_spmd

    x = np.asarray(x, dtype=np.float32)
    Wr = np.asarray(Wr, dtype=np.float32)
    br = np.asarray(br, dtype=np.float32)
    Wg = np.asarray(Wg, dtype=np.float32)
    Wu = np.asarray(Wu, dtype=np.float32)
    Wd = np.asarray(Wd, dtype=np.float32)

    topk_idx, g = _route(x, Wr, br)

    # Per-expert token lists
    idx_lists = []
    gate_lists = []
    for e in range(E):
        mask = topk_idx == e                    # [T, K]
        tok = np.nonzero(mask.any(axis=1))[0]
        gsel = np.where(mask[tok, 0], g[tok, 0], g[tok, 1]).astype(np.float32)
        idx_lists.append(tok.astype(np.int64))
        gate_lists.append(gsel)

    counts = [len(ix) for ix in idx_lists]
    C = max(512, max(counts))

    key = C
    if key not in _compiled:
        _compiled[key] = _build(C)
    nc = _compiled[key]

    bf16 = ml_dtypes.bfloat16
    xTb = np.ascontiguousarray(x.T).astype(bf16)   # [H, T]

    def _wR(w):
        # [H, I] -> [PB, IB*KB*PB] with col (ib*KB + k)*PB + c
        return np.ascontiguousarray(
            w.reshape(KB, PB, IB, PB).transpose(1, 2, 0, 3).reshape(PB, -1)
        ).astype(bf16)

    def _wdR(w):
        # [I, H] -> [PB, IB*H] with col ib*H + h
        return np.ascontiguousarray(
            w.reshape(IB, PB, H).transpose(1, 0, 2).reshape(PB, -1)
        ).astype(bf16)

    in_maps = []
    for e in range(E):
        n = counts[e]
        xTe = np.zeros((H, C), dtype=bf16)
        xTe[:, :n] = xTb[:, idx_lists[e]]
        gme = np.zeros((PB, C), dtype=np.float32)
        gme[:, :n] = gate_lists[e][None, :]
        in_maps.append({
            "xT": xTe,
            "gm": gme,
            "Wg": _wR(Wg[e]),
            "Wu": _wR(Wu[e]),
            "Wd": _wdR(Wd[e]),
        })

    trace = bool(int(os.environ.get("MOE_TRACE", "0")))
    trace_cores = (list(range(NCORES))
                   if os.environ.get("MOE_TRACE_ALL") else None)
    last_results = run_bass_kernel_spmd(
        nc, in_maps, core_ids=list(range(NCORES)), trace=trace,
        trace_cores=trace_cores)

    out = np.zeros((T, H), dtype=np.float32)
    for e in range(E):
        n = counts[e]
        yTe = last_results.results[e]["yT"]
        out[idx_lists[e]] += yTe[:, :n].T
    return out


# revision 34
# speedup vs baseline: 1.0475x; 1.0475x over previous
"""MoE (top-2 of 8 experts, SwiGLU) Trainium2 kernel — fused bf16 single pass.

Strategy (expert-parallel over 8 NeuronCores):
  * Host: router GEMM + top-2 + sigmoid gates in numpy (selection verified to
    match the jax fp32 reference on these inputs), then gather each expert's
    tokens into a transposed, capacity-padded bf16 buffer xT_e [H, C]. One
    expert per core, capacity C = max_e count_e.
  * Device (SPMD, per core): all three weight matrices live in SBUF as bf16
    (12 MB total), x and the gates are SBUF-resident too.  Tokens are
    processed in segments of <=512 (the PSUM free-dim limit); for each
    segment the SwiGLU intermediate h = silu(x@Wg) * (x@Wu) is produced
    i-block by i-block into SBUF (bf16) and immediately consumed by the
    down-projection y = (h@Wd) * gate — h never leaves the chip.
    The down-projection runs hb-outer so only 2 PSUM banks are needed and
    eviction is progressive (short kernel tail).
  * bf16 matmuls stream at the same 1 cycle/row as fp32r but halve SBUF and
    HBM traffic and enable fast weight loads; accumulation is fp32 in PSUM.
    The measured output error vs the fp64 reference is ~3e-3.
  * A burst of warm-up matmuls on the first x tile runs while weights are
    still streaming in, so the PE's HAM clock gate reaches full rate before
    the real matmul stream starts.
  * Host: out[idx_e] += yT_e[:, :n_e].T  (indices within one expert are
    unique, so fancy-index += is safe).
"""

import os
import numpy as np
import ml_dtypes

T, H, I, E, TOPK = 8192, 1024, 2048, 8, 2
NCORES = 8
PB = 128
KB = H // PB   # 8 contraction blocks over H
IB = I // PB   # 16 blocks over I
HB = H // PB   # 8 output blocks over H

# Wg/Wu i-block chunks (need-ordered streaming, one DMA per chunk).
# Host re-lays Wg/Wu as WgR[r, (ib*KB + k)*PB + c] = Wg[k*PB + r, ib*PB + c]
# so that any i-block range for ALL k-blocks is one contiguous DMA.
WCH = [(0, 1), (1, 2), (2, 4), (4, 8), (8, 12), (12, 16)]
DCH = [(0, 8), (8, 16)]   # Wd i-block chunks (WdR layout, see below)

_compiled = {}
last_results = None  # BassKernelResults of the most recent run (for test harness)


def _tsegs(C):
    """Split C into segments of width 256..512."""
    widths = []
    rem = C
    while rem >= 768:
        widths.append(512)
        rem -= 512
    if rem <= 512:
        widths.append(rem)
    else:
        widths.append(rem - 256)
        widths.append(256)
    segs = []
    t0 = 0
    for tb in widths:
        segs.append((t0, tb))
        t0 += tb
    return segs


def _build(C):
    import concourse.bacc as bacc
    import concourse.mybir as mybir
    import concourse.tile as tile
    from contextlib import ExitStack

    fp32 = mybir.dt.float32
    bf16 = mybir.dt.bfloat16
    AF = mybir.ActivationFunctionType

    segs = _tsegs(C)
    s0w = segs[0][1]

    nc = bacc.Bacc("TRN2", target_bir_lowering=False, debug=False,
                   num_devices=NCORES)
    xT = nc.dram_tensor("xT", [H, C], bf16, kind="ExternalInput").ap()
    gm = nc.dram_tensor("gm", [PB, C], fp32, kind="ExternalInput").ap()
    Wg = nc.dram_tensor("Wg", [PB, IB * KB * PB], bf16,
                        kind="ExternalInput").ap()
    Wu = nc.dram_tensor("Wu", [PB, IB * KB * PB], bf16,
                        kind="ExternalInput").ap()
    Wd = nc.dram_tensor("Wd", [PB, IB * H], bf16, kind="ExternalInput").ap()
    yT = nc.dram_tensor("yT", [H, C], fp32, kind="ExternalOutput").ap()

    with tile.TileContext(nc) as tc, ExitStack() as st:
        wp = st.enter_context(tc.tile_pool(name="wp", bufs=1))
        hp = st.enter_context(tc.tile_pool(name="hp", bufs=2))
        ev1 = st.enter_context(tc.tile_pool(name="ev1", bufs=2))
        ev2 = st.enter_context(tc.tile_pool(name="ev2", bufs=3))
        ps1 = st.enter_context(tc.tile_pool(name="ps1", bufs=2, space="PSUM"))
        ps2 = st.enter_context(tc.tile_pool(name="ps2", bufs=4, space="PSUM"))

        # ---- load issue order.  The critical stream (x seg0, then Wg/Wu in
        # i-block need-order) is split between the sync and gpsimd queues;
        # everything needed later (gates, Wd, x remainder) goes on the scalar
        # queue, paced behind the per-i-block silu ops so it cannot steal
        # bandwidth from the critical window. ----
        # Warm-up matmuls on a memset scratch tile (no DMA dependency): the
        # PE is busy from ~7us — right after the framework preamble — so the
        # HAM clock gate reaches 8/8 before the real stream starts, and the
        # PE has work while the first weight chunks land.  They write
        # rotating ps2 slots, long retired before phase 2 reuses them.
        wscr = wp.tile([PB, 512], bf16, name="wscr")
        nc.gpsimd.memset(wscr[:], 0.0)
        for i in range(12):
            pwarm = ps2.tile([PB, 512], fp32, tag="py", name="py")
            nc.tensor.matmul(pwarm[:], wscr[:, 0:PB], wscr[:],
                             start=True, stop=True)

        # Segment-0 x: per-k tiles spread over all three queues so arrival
        # granularity is fine (chains pipeline with landings).
        xq = [nc.sync, nc.gpsimd, nc.scalar]
        xs0 = []
        for k in range(KB):
            t = wp.tile([PB, s0w], bf16, name=f"xs0_{k}")
            xq[k % 3].dma_start(out=t[:], in_=xT[k * PB:(k + 1) * PB, 0:s0w])
            xs0.append(t)

        # Wg/Wu in need-ordered i-block chunks, one DMA each, alternating
        # sync/gpsimd so both queues carry half of the critical stream.
        IBW = KB * PB   # column span of one i-block in the WgR/WuR layout
        wgt, wut = [], []
        for c, (a, b) in enumerate(WCH):
            qa, qb = (nc.sync, nc.gpsimd) if c % 2 == 0 \
                else (nc.gpsimd, nc.sync)
            t = wp.tile([PB, (b - a) * IBW], bf16, name=f"wg{c}")
            qa.dma_start(out=t[:], in_=Wg[:, a * IBW:b * IBW])
            wgt.append(t)
            t = wp.tile([PB, (b - a) * IBW], bf16, name=f"wu{c}")
            qb.dma_start(out=t[:], in_=Wu[:, a * IBW:b * IBW])
            wut.append(t)

        # Late loads (gates, Wd, x remainder): issued at the BACK of the sync
        # and gpsimd queues.  In-queue FIFO ordering paces their transfers
        # behind the critical Wg/Wu stream — the Tile scheduler would hoist
        # them if they sat dep-free on an otherwise-busy engine.
        gt = wp.tile([PB, C], fp32, name="gt")
        wdt = [wp.tile([PB, (b - a) * H], bf16, name=f"wd{c}")
               for c, (a, b) in enumerate(DCH)]
        xr = [wp.tile([PB, C - s0w], bf16, name=f"xr{k}") for k in range(KB)] \
            if C > s0w else []
        nc.gpsimd.dma_start(out=gt[:], in_=gm[:])
        for c, (a, b) in enumerate(DCH):
            q = nc.sync if c % 2 == 0 else nc.gpsimd
            q.dma_start(out=wdt[c][:], in_=Wd[:, a * H:b * H])
        for k in range(len(xr)):
            q = nc.sync if k % 2 == 0 else nc.gpsimd
            q.dma_start(out=xr[k][:], in_=xT[k * PB:(k + 1) * PB, s0w:C])

        def _chunk(ch, ib):
            for j, (a, b) in enumerate(ch):
                if ib < b:
                    return j, ib - a
            raise AssertionError

        def wg_sl(k, ib):
            j, off = _chunk(WCH, ib)
            return wgt[j][:, (off * KB + k) * PB:(off * KB + k) * PB + PB]

        def wu_sl(k, ib):
            j, off = _chunk(WCH, ib)
            return wut[j][:, (off * KB + k) * PB:(off * KB + k) * PB + PB]

        def wd_sl(ib, hb):
            j, off = _chunk(DCH, ib)
            return wdt[j][:, off * H + hb * PB:off * H + hb * PB + PB]

        def x_sl(k, t0, w):
            if t0 >= s0w:
                return xr[k][:, t0 - s0w:t0 - s0w + w]
            return xs0[k][:, t0:t0 + w]

        for si, (t0, w) in enumerate(segs):
            last_seg = si == len(segs) - 1
            hts = []
            for ib in range(IB):
                pg = ps1.tile([PB, w], fp32, tag="pg", name="pg")
                pu = ps1.tile([PB, w], fp32, tag="pu", name="pu")
                for k in range(KB):
                    nc.tensor.matmul(pg[:], wg_sl(k, ib), x_sl(k, t0, w),
                                     start=(k == 0), stop=(k == KB - 1))
                for k in range(KB):
                    nc.tensor.matmul(pu[:], wu_sl(k, ib), x_sl(k, t0, w),
                                     start=(k == 0), stop=(k == KB - 1))
                sg = ev1.tile([PB, w], fp32, tag="sg", name="sg")
                nc.scalar.activation(sg[:], pg[:], AF.Silu)
                hh = hp.tile([PB, w], bf16, tag=f"h{ib}", name=f"h{ib}")
                nc.vector.tensor_mul(hh[:], sg[:], pu[:])
                hts.append(hh)

            for hb in range(HB):
                py = ps2.tile([PB, w], fp32, tag="py", name="py")
                for ib in range(IB):
                    nc.tensor.matmul(py[:], wd_sl(ib, hb), hts[ib][:],
                                     start=(ib == 0), stop=(ib == IB - 1))
                yt = ev2.tile([PB, w], fp32, tag="yt", name="yt")
                nc.vector.tensor_mul(yt[:], py[:], gt[:, t0:t0 + w])
                eng = nc.sync if last_seg else nc.gpsimd
                eng.dma_start(out=yT[hb * PB:(hb + 1) * PB, t0:t0 + w],
                              in_=yt[:])
    nc.compile()
    return nc


def _route(x, Wr, br):
    """Replicate the reference's fp32 router bit-compatibly on host."""
    logits = x @ Wr + br                       # fp32 GEMM
    order = np.argsort(-logits, axis=1, kind="stable")  # ties -> lowest index
    topk_idx = order[:, :TOPK]
    topk_vals = np.take_along_axis(logits, topk_idx, axis=1)
    g = 1.0 / (1.0 + np.exp(-topk_vals.astype(np.float32)))
    g = g / (np.sum(g, axis=-1, keepdims=True) + 1e-10)
    return topk_idx, g.astype(np.float32)


def kernel(x, Wr, br, Wg, Wu, Wd):
    global last_results
    from concourse.bass_utils import run_bass_kernel_spmd

    x = np.asarray(x, dtype=np.float32)
    Wr = np.asarray(Wr, dtype=np.float32)
    br = np.asarray(br, dtype=np.float32)
    Wg = np.asarray(Wg, dtype=np.float32)
    Wu = np.asarray(Wu, dtype=np.float32)
    Wd = np.asarray(Wd, dtype=np.float32)

    topk_idx, g = _route(x, Wr, br)

    # Per-expert token lists
    idx_lists = []
    gate_lists = []
    for e in range(E):
        mask = topk_idx == e                    # [T, K]
        tok = np.nonzero(mask.any(axis=1))[0]
        gsel = np.where(mask[tok, 0], g[tok, 0], g[tok, 1]).astype(np.float32)
        idx_lists.append(tok.astype(np.int64))
        gate_lists.append(gsel)

    counts = [len(ix) for ix in idx_lists]
    C = max(512, max(counts))

    key = C
    if key not in _compiled:
        _compiled[key] = _build(C)
    nc = _compiled[key]

    bf16 = ml_dtypes.bfloat16
    xTb = np.ascontiguousarray(x.T).astype(bf16)   # [H, T]

    def _wR(w):
        # [H, I] -> [PB, IB*KB*PB] with col (ib*KB + k)*PB + c
        return np.ascontiguousarray(
            w.reshape(KB, PB, IB, PB).transpose(1, 2, 0, 3).reshape(PB, -1)
        ).astype(bf16)

    def _wdR(w):
        # [I, H] -> [PB, IB*H] with col ib*H + h
        return np.ascontiguousarray(
            w.reshape(IB, PB, H).transpose(1, 0, 2).reshape(PB, -1)
        ).astype(bf16)

    in_maps = []
    for e in range(E):
        n = counts[e]
        xTe = np.zeros((H, C), dtype=bf16)
        xTe[:, :n] = xTb[:, idx_lists[e]]
        gme = np.zeros((PB, C), dtype=np.float32)
        gme[:, :n] = gate_lists[e][None, :]
        in_maps.append({
            "xT": xTe,
            "gm": gme,
            "Wg": _wR(Wg[e]),
            "Wu": _wR(Wu[e]),
            "Wd": _wdR(Wd[e]),
        })

    trace = bool(int(os.environ.get("MOE_TRACE", "0")))
    trace_cores = (list(range(NCORES))
                   if os.environ.get("MOE_TRACE_ALL") else None)
    last_results = run_bass_kernel_spmd(
        nc, in_maps, core_ids=list(range(NCORES)), trace=trace,
        trace_cores=trace_cores)

    out = np.zeros((T, H), dtype=np.float32)
    for e in range(E):
        n = counts[e]
        yTe = last_results.results[e]["yT"]
        out[idx_lists[e]] += yTe[:, :n].T
    return out
